# revision 10
# baseline (speedup 1.0000x reference)
"""Distance-correlation (DisCo) loss kernel for Trainium2, sharded over 8 NeuronCores.

Math: reference computes NxN pairwise |vi-vj| matrices (a, b), weighted row
means, double-centering, then scalar reductions.  Everything except the
genuinely 2-D term

    Q_ab[i] = sum_j w_j * |v1_i - v1_j| * |v2_i - v2_j|

has an exact O(N log N) closed form on the host (sorted prefix sums for
weighted |.| row sums, polynomial identities for squared terms).  The device
computes Q_ab only, with rows i sharded across the 8 cores (1024 rows/core).

Device layout (per core): i on partitions (8 blocks of 128), j on the free
dim (2 chunks of 4096).  Tiles a=|v1_i-v1_j| and b=|v2_i-v2_j| are built by
the Vector engine (tensor_scalar: abs_max(in0-s, 0), fp32 2x mode) and the
Scalar engine (activation Abs with per-partition bias) splitting the j
columns, then a fused tensor_tensor_reduce (mult + free-dim add) produces
the row sums, chained across j chunks.  The weighted fallback multiplies b
by a broadcast w tile first.
"""

import functools
import os

import numpy as np

N = 8192
CORES = 8
ROWS = N // CORES          # 1024 rows per core
NIB = ROWS // 128          # 8 partition blocks per core
BCH = 1024                 # broadcast DMA chunk

LAST_RESULT = None         # BassKernelResults of the most recent launch


@functools.lru_cache(maxsize=2)
def _build(weighted: bool):
    import concourse.bacc as bacc
    import concourse.bass as bass
    import concourse.tile as tile
    from concourse import mybir

    f32 = mybir.dt.float32
    nc = bacc.Bacc("TRN2", target_bir_lowering=False, debug=False)

    # j-chunk size and the VectorE share of build columns, chosen to balance
    # VectorE vs ScalarE busy time per chunk while fitting SBUF.
    JC = 2048 if weighted else 4096
    JD = 0 if weighted else 512
    NJC = N // JC

    v1d = nc.dram_tensor("v1", [N], f32, kind="ExternalInput")
    v2d = nc.dram_tensor("v2", [N], f32, kind="ExternalInput")
    wd = nc.dram_tensor("w", [N], f32, kind="ExternalInput") if weighted else None
    # vipack columns: [vi1 | -vi1 | vi2 | -vi2], each NIB wide, partition-major.
    vipackd = nc.dram_tensor("vipack", [128, 4 * NIB], f32, kind="ExternalInput")
    qabd = nc.dram_tensor("qab", [128, NIB], f32, kind="ExternalOutput")

    def bcast(ap1d):
        return bass.AP(
            tensor=ap1d.tensor, offset=ap1d.offset, ap=[[0, 128]] + list(ap1d.ap)
        )

    i32 = mybir.dt.int32
    sub = mybir.AluOpType.subtract
    band = mybir.AluOpType.bitwise_and
    mult = mybir.AluOpType.mult
    add = mybir.AluOpType.add

    with tile.TileContext(nc) as tc:
        with (
            tc.tile_pool(name="singles", bufs=1) as singles,
            tc.tile_pool(name="ab", bufs=2) as pab,
            tc.tile_pool(name="scrap", bufs=1) as pscrap,
        ):
            v1rep = singles.tile([128, N], f32)
            v2rep = singles.tile([128, N], f32)
            reps = [(v1rep, v1d), (v2rep, v2d)]
            wrep = None
            if weighted:
                wrep = singles.tile([128, N], f32)
                reps.append((wrep, wd))
            for rep, src in reps:
                sap = src.ap()
                for c in range(N // BCH):
                    nc.sync.dma_start(
                        out=rep[:, c * BCH : (c + 1) * BCH],
                        in_=bcast(sap[c * BCH : (c + 1) * BCH]),
                    )

            vipack = singles.tile([128, 4 * NIB], f32)
            nc.sync.dma_start(out=vipack[:, :], in_=vipackd.ap())
            vi1 = vipack[:, 0 * NIB : 1 * NIB]
            nvi1 = vipack[:, 1 * NIB : 2 * NIB]
            vi2 = vipack[:, 2 * NIB : 3 * NIB]
            nvi2 = vipack[:, 3 * NIB : 4 * NIB]

            qacc = singles.tile([128, NIB], f32)

            mask = None
            if JD > 0:
                # 0x7FFFFFFF sign-clear mask: |x| on VectorE is a fp32
                # subtract followed by an int32 bitwise_and against this.
                mask = singles.tile([128, JD], i32)
                nc.vector.memset(mask, 0x7FFFFFFF)

            for ib in range(NIB):
                for jc in range(NJC):
                    j0 = jc * JC
                    ab = pab.tile([128, 2, JC], f32, tag="ab")
                    a = ab[:, 0, :]
                    b = ab[:, 1, :]
                    for t, (rep, vis, nvis) in enumerate(
                        ((v1rep, vi1, nvi1), (v2rep, vi2, nvi2))
                    ):
                        if JD > 0:
                            nc.vector.tensor_scalar(
                                ab[:, t, :JD],
                                rep[:, j0 : j0 + JD],
                                vis[:, ib : ib + 1],
                                None,
                                sub,
                            )
                        nc.scalar.activation(
                            out=ab[:, t, JD:],
                            in_=rep[:, j0 + JD : j0 + JC],
                            func=mybir.ActivationFunctionType.Abs,
                            bias=nvis[:, ib : ib + 1],
                            scale=1.0,
                        )
                    if JD > 0:
                        for t in range(2):
                            nc.vector.tensor_tensor(
                                ab[:, t, :JD].bitcast(i32),
                                ab[:, t, :JD].bitcast(i32),
                                mask[:, :],
                                band,
                            )
                    rhs = b
                    if weighted:
                        wb = pab.tile([128, JC], f32, tag="wb")
                        nc.vector.tensor_tensor(wb, b, wrep[:, j0 : j0 + JC], mult)
                        rhs = wb
                    scrap = pscrap.tile([128, JC], f32)
                    nc.vector.tensor_tensor(scrap, a, rhs, mult)
                    # in-place copy whose op1 performs the free-dim reduction,
                    # chained across j chunks via the scalar2 initializer
                    nc.vector.tensor_scalar(
                        scrap,
                        scrap,
                        1.0,
                        (0.0 if jc == 0 else qacc[:, ib : ib + 1]),
                        mult,
                        add,
                        accum_out=qacc[:, ib : ib + 1],
                    )

            nc.sync.dma_start(out=qabd.ap(), in_=qacc[:, :])

    nc.compile()
    return nc


def _abs_weighted_sums(q, x):
    """out_i = sum_j q_j * |x_i - x_j|, exact via sorting (float64)."""
    o = np.argsort(x, kind="stable")
    xs, qs = x[o], q[o]
    cq = np.cumsum(qs)
    cqx = np.cumsum(qs * xs)
    vals = xs * (2.0 * cq - cq[-1]) + cqx[-1] - 2.0 * cqx
    out = np.empty_like(vals)
    out[o] = vals
    return out


def _run_device_qab(v1, v2, w, ones):
    from concourse.bass_utils import run_bass_kernel_spmd

    global LAST_RESULT
    nc = _build(not ones)
    trace = os.environ.get("DISCO_TRACE", "0") == "1"
    in_maps = []
    for c in range(CORES):
        vr1 = v1[c * ROWS : (c + 1) * ROWS].reshape(NIB, 128).T
        vr2 = v2[c * ROWS : (c + 1) * ROWS].reshape(NIB, 128).T
        m = {
            "v1": v1,
            "v2": v2,
            "vipack": np.ascontiguousarray(
                np.concatenate([vr1, -vr1, vr2, -vr2], axis=1)
            ),
        }
        if not ones:
            m["w"] = w
        in_maps.append(m)
    res = run_bass_kernel_spmd(
        nc, in_maps, core_ids=list(range(CORES)), trace=trace
    )
    LAST_RESULT = res
    return np.concatenate(
        [r["qab"].T.reshape(ROWS) for r in res.results]
    ).astype(np.float64)


def kernel(var_1, var_2, normedweight, power):
    v1 = np.ascontiguousarray(np.asarray(var_1, dtype=np.float32))
    v2 = np.ascontiguousarray(np.asarray(var_2, dtype=np.float32))
    w = np.ascontiguousarray(np.asarray(normedweight, dtype=np.float32))
    p = int(np.asarray(power))
    ones = bool(np.all(w == np.float32(1.0)))

    qab = _run_device_qab(v1, v2, w, ones)

    v1d, v2d, wd = v1.astype(np.float64), v2.astype(np.float64), w.astype(np.float64)
    u = _abs_weighted_sums(wd, v1d) / N
    v = _abs_weighted_sums(wd, v2d) / N
    W = wd.sum()
    ga = (wd * u).mean()
    gb = (wd * v).mean()
    al = u - ga
    be = v - gb
    Qaa = W * v1d**2 - 2.0 * v1d * (wd * v1d).sum() + (wd * v1d**2).sum()
    Qbb = W * v2d**2 - 2.0 * v2d * (wd * v2d).sum() + (wd * v2d**2).sum()
    Duu = (wd * u * u).sum()
    Duv = (wd * u * v).sum()
    Dvv = (wd * v * v).sum()
    Rawu = _abs_weighted_sums(wd * u, v1d)
    Rawv = _abs_weighted_sums(wd * v, v1d)
    Rbwu = _abs_weighted_sums(wd * u, v2d)
    Rbwv = _abs_weighted_sums(wd * v, v2d)

    k = 2.0 * N - W
    SAA = Qaa - 2.0 * Rawu + Duu - al**2 * k
    SBB = Qbb - 2.0 * Rbwv + Dvv - be**2 * k
    SAB = qab - Rawv - Rbwu + Duv - al * be * k

    num = (np.abs(SAB) / N * wd).mean()
    denA = (SAA / N * wd).mean()
    denB = (SBB / N * wd).mean()
    EPS = 1e-12
    with np.errstate(all="ignore"):
        if p == 1:
            d = np.abs(denA * denB)
            out = num / np.sqrt(d + EPS)
        elif p == 2:
            d = np.abs(denA * denB)
            out = num**2 / (d + EPS)
        else:
            out = (num / np.sqrt(denA * denB) + EPS) ** p
    if np.isnan(out):
        out = 0.0
    out = max(out, 0.0)
    return np.float32(out)
